# revision 12
# baseline (speedup 1.0000x reference)
"""Multi-head attention (B=2, N=4096, D=512, H=8) on 8 trn2 NeuronCores.

Sharding: core c handles batch b = c//4 and head-pair p = c%4 (heads 2p,
2p+1).  Each core projects its batch's Q/K against its pair's weight
columns (q^T/k^T layout, dk on partitions), projects V directly into
[keys, dk] layout (lhsT = X chunk), computes transposed attention scores
sT = K_h @ Q_h^T in bf16, applies exp((1/8)*sT) on the ACT engine (fp16
out), and multiplies by an augmented V (extra ones column per head) so
the softmax denominators fall out of the same matmul.

The output projection commutes with the per-query softmax normalization
(it is a row scale), so both are done on the host: the device returns
unnormalized O^T per head ([65, N]: 64 value dims + denominator row).

Engine budget per core (the kernel is ACT-bound):
  ACT: 176 exp instructions over [128,1536] tiles  (~251 us model time)
  PE : scores 109 + PV 109 + proj 20 us
  DVE: all psum->sbuf epilogue copies (~30 us)
"""

import numpy as np

_B, _N, _D, _H, _DK = 2, 4096, 512, 8, 64
_NCORES = 8

_nc_cache = {}


def _build(n=_N, zero_bias=False):
    import concourse.mybir as mybir
    import concourse.tile as tile
    from concourse import bacc

    f32 = mybir.dt.float32
    f16 = mybir.dt.float16
    Exp = mybir.ActivationFunctionType.Exp
    D = _D
    NKC = n // 128  # key chunks of 128
    NQC = n // 512  # query chunks of 512
    blocks = []
    i = 0
    while i < NKC:
        blen = min(3, NKC - i)
        blocks.append((i, blen))
        i += blen

    nc = bacc.Bacc(
        "TRN2", target_bir_lowering=False, debug=False, num_devices=_NCORES
    )

    # X^T pre-permuted on host to [128, 4, n]: (p, dc, j) = X^T[dc*128+p, j]
    xt = {
        t: nc.dram_tensor(f"xt{t}", [128, 4, n], f16, kind="ExternalInput").ap()
        for t in "qkv"
    }
    w = {
        t: nc.dram_tensor(f"w{t}", [128, 4, 128], f16, kind="ExternalInput").ap()
        for t in "qkv"
    }
    bvec = {}
    if not zero_bias:
        for t in "qk":
            bvec[t] = nc.dram_tensor(f"b{t}", [128, 1], f32, kind="ExternalInput").ap()
        # v bias broadcast across partitions: [128, 128] (pair slice, both heads)
        bvec["v"] = nc.dram_tensor("bv", [128, 128], f32, kind="ExternalInput").ap()
    o_out = [
        nc.dram_tensor(f"o{h}", [65, n], f32, kind="ExternalOutput").ap()
        for h in range(2)
    ]

    with tile.TileContext(nc) as tc:
        with (
            tc.tile_pool(name="consts", bufs=1) as consts,
            tc.tile_pool(name="xtp", bufs=3) as xtp,
            tc.tile_pool(name="persist", bufs=1) as persist,
            tc.tile_pool(name="ep", bufs=4) as ep,
            tc.tile_pool(name="osb", bufs=3) as osb,
            tc.tile_pool(name="psA", bufs=2, space="PSUM") as psA,
            tc.tile_pool(name="psB", bufs=2, space="PSUM") as psB,
        ):
            wsb, bsb = {}, {}
            for t in "qkv":
                wsb[t] = consts.tile([128, 4, 128], f16, name=f"w{t}sb", tag=f"w{t}sb")
                nc.sync.dma_start(out=wsb[t], in_=w[t])
                if not zero_bias:
                    shape = [128, 128] if t == "v" else [128, 1]
                    bsb[t] = consts.tile(shape, f32, name=f"b{t}sb", tag=f"b{t}sb")
                    nc.sync.dma_start(out=bsb[t], in_=bvec[t])

            qt_t = [
                persist.tile([128, 512], f16, name=f"qt{i}", tag=f"qt{i}")
                for i in range(NQC)
            ]
            kt_t = [
                persist.tile([128, 512], f16, name=f"kt{i}", tag=f"kt{i}")
                for i in range(NQC)
            ]
            # augmented V per key-chunk: [keys, h0 dims | h0 ones | h1 dims | h1 ones]
            vch = [
                persist.tile([128, 130], f16, name=f"vch{c}", tag=f"vch{c}")
                for c in range(NKC)
            ]
            for c in range(NKC):
                nc.gpsimd.memset(vch[c][:, 64:65], 1.0)
                nc.gpsimd.memset(vch[c][:, 129:130], 1.0)

            # ---- phase 1: projections ----
            # k^T/q^T: [dk-pair on partitions, seq]  (t^T = W_p^T @ X^T)
            def kq_proj(t, dest, nk, dma_eng):
                xtile = xtp.tile([128, 4, 512], f16, name=f"x_{t}{nk}", tag="xt")
                dma_eng.dma_start(
                    out=xtile, in_=xt[t][:, :, nk * 512 : (nk + 1) * 512]
                )
                ppsum = psA.tile([128, 512], f32, name=f"pp_{t}{nk}", tag="s")
                for dc in range(4):
                    nc.tensor.matmul(
                        ppsum,
                        wsb[t][:, dc, :],
                        xtile[:, dc, :],
                        start=(dc == 0),
                        stop=(dc == 3),
                    )
                if zero_bias:
                    nc.vector.tensor_copy(out=dest[nk], in_=ppsum)
                else:
                    nc.vector.tensor_scalar_add(
                        out=dest[nk], in0=ppsum, scalar1=bsb[t]
                    )

            # v directly in [keys, dk-pair] layout: lhsT = X chunk
            def v_proj(nk):
                xtile = xtp.tile([128, 4, 512], f16, name=f"x_v{nk}", tag="xt")
                nc.gpsimd.dma_start(
                    out=xtile, in_=xt["v"][:, :, nk * 512 : (nk + 1) * 512]
                )
                for ck in range(4):
                    c = nk * 4 + ck
                    vpsum = psB.tile([128, 128], f32, name=f"vp{c}", tag="oy")
                    for dc in range(4):
                        nc.tensor.matmul(
                            vpsum,
                            xtile[:, dc, ck * 128 : (ck + 1) * 128],
                            wsb["v"][:, dc, :],
                            start=(dc == 0),
                            stop=(dc == 3),
                        )
                    for h in range(2):
                        dst = vch[c][:, h * 65 : h * 65 + 64]
                        src = vpsum[:, h * 64 : (h + 1) * 64]
                        if zero_bias:
                            nc.vector.tensor_copy(out=dst, in_=src)
                        else:
                            nc.vector.tensor_tensor(
                                out=dst, in0=src,
                                in1=bsb["v"][:, h * 64 : (h + 1) * 64],
                                op=mybir.AluOpType.add,
                            )

            for nk in range(NQC):
                kq_proj("k", kt_t, nk, nc.sync)
            kq_proj("q", qt_t, 0, nc.gpsimd)
            v_done = 0  # V chunks projected so far (in units of nk)

            # ---- phase 2: attention (V and later-Q projections interleaved
            # into the first iterations so the ACT engine starts early) ----
            for qc in range(NQC):
                if qc + 1 < NQC:
                    kq_proj("q", qt_t, qc + 1, nc.gpsimd)
                for h in range(2):
                    hp = slice(h * 64, (h + 1) * 64)
                    o_ps = psB.tile([65, 512], f32, name=f"o_{h}_{qc}", tag="oy")
                    for k0, blen in blocks:
                        s_ps = psA.tile(
                            [128, blen * 512], f32, name=f"s_{h}_{qc}_{k0}", tag="s"
                        )
                        for j in range(blen):
                            kc = k0 + j
                            nc.tensor.matmul(
                                s_ps[:, j * 512 : (j + 1) * 512],
                                kt_t[kc // 4][hp, (kc % 4) * 128 : (kc % 4 + 1) * 128],
                                qt_t[qc][hp, :],
                                start=True,
                                stop=True,
                                skip_group_check=True,
                            )
                        # keep V projection just ahead of the PV consumers
                        nk_needed = min((k0 + blen + 3) // 4 + 1, NQC)
                        while v_done < nk_needed:
                            v_proj(v_done)
                            v_done += 1
                        e_sb = ep.tile(
                            [128, blen * 512], f16, name=f"e_{h}_{qc}_{k0}", tag="e"
                        )
                        nc.scalar.activation(e_sb, s_ps, Exp, scale=0.125)
                        for j in range(blen):
                            kc = k0 + j
                            nc.tensor.matmul(
                                o_ps,
                                vch[kc][:, h * 65 : (h + 1) * 65],
                                e_sb[:, j * 512 : (j + 1) * 512],
                                start=(kc == 0),
                                stop=(kc == NKC - 1),
                                skip_group_check=True,
                            )
                    o_sb = osb.tile([65, 512], f32, name=f"ob_{h}_{qc}", tag="osb")
                    nc.vector.tensor_copy(out=o_sb, in_=o_ps)
                    nc.sync.dma_start(
                        out=o_out[h][:, qc * 512 : (qc + 1) * 512], in_=o_sb
                    )
    nc.finalize()
    return nc


def get_nc(n=_N, zero_bias=False):
    key = (n, zero_bias)
    if key not in _nc_cache:
        _nc_cache[key] = _build(n, zero_bias)
    return _nc_cache[key]


def make_in_maps(Q, K, V, Wq, bq, Wk, bk, Wv, bv, Wo, bo, n=_N, zero_bias=True):
    """Per-core input dicts (host-side sharding / layout prep)."""
    hf = np.float16
    perm = lambda X: np.ascontiguousarray(
        X[:n].T.reshape(4, 128, n).transpose(1, 0, 2).astype(hf)
    )
    xts = {}
    for b in range(_B):
        xts[b] = {"xtq": perm(Q[b]), "xtk": perm(K[b]), "xtv": perm(V[b])}
    in_maps = []
    for c in range(_NCORES):
        b, p = divmod(c, 4)
        off = p * 128
        m = dict(xts[b])
        for t, W in (("q", Wq), ("k", Wk), ("v", Wv)):
            m[f"w{t}"] = np.ascontiguousarray(
                W[:, off : off + 128].reshape(4, 128, 128).transpose(1, 0, 2).astype(hf)
            )
        if not zero_bias:
            m["bq"] = np.ascontiguousarray(bq[off : off + 128].reshape(128, 1))
            m["bk"] = np.ascontiguousarray(bk[off : off + 128].reshape(128, 1))
            m["bv"] = np.ascontiguousarray(
                np.broadcast_to(bv[off : off + 128], (128, 128)).astype(np.float32)
            )
        in_maps.append(m)
    return in_maps


def assemble(results, Wo, bo, n=_N):
    """Host: normalize by softmax denominators, apply output projection,
    sum heads, add output bias."""
    Wo = np.asarray(Wo, np.float32)
    out = np.zeros((_B, n, _D), np.float32)
    for c in range(_NCORES):
        b, p = divmod(c, 4)
        r = results[c]
        for h in range(2):
            arr = r[f"o{h}"]              # [65, n]
            den = arr[64]                 # [n]
            On = (arr[0:64] / den).T      # [n, 64]
            hg = 2 * p + h
            out[b] += On @ Wo[hg * 64 : (hg + 1) * 64]
    out += bo
    return out


def kernel(Q, K, V, Wq, bq, Wk, bk, Wv, bv, Wo, bo):
    from concourse import bass_utils

    args = [np.asarray(a, np.float32) for a in (Q, K, V, Wq, bq, Wk, bk, Wv, bv, Wo, bo)]
    Q, K, V, Wq, bq, Wk, bk, Wv, bv, Wo, bo = args
    zb = not (np.any(bq) or np.any(bk) or np.any(bv))
    nc = get_nc(zero_bias=zb)
    in_maps = make_in_maps(Q, K, V, Wq, bq, Wk, bk, Wv, bv, Wo, bo, zero_bias=zb)
    res = bass_utils.run_bass_kernel_spmd(
        nc, in_maps, core_ids=list(range(_NCORES))
    )
    return assemble(res.results, Wo, bo)
